# revision 8
# baseline (speedup 1.0000x reference)
"""Multi-head attention (B=4, N=2048, C=256, H=8) on 8 Trainium2 NeuronCores.

Sharding: core c handles batch b = c//2 and query-half qh = c%2 (1024 query
rows), all 8 heads. k/v are computed for the full sequence on each core (the
qkv projection is cheap); outputs concatenate with no cross-core reduction.

Device-side layout is fully "transposed" (channels on partitions):
  - x^T [C, N] feeds q^T/k^T ([d, tokens], head-major rows) and v ([tokens, d]).
  - Scores are computed as S^T [k-tokens, q-tokens] so that softmax's exp
    output E^T feeds the AV matmul directly (contraction over k on partitions).
  - Softmax denominators come for free as a 33rd "ones" column appended to v.
  - O^T [channels, q] feeds the output projection directly.

Softmax exp is split across TWO engines running concurrently:
  - The q-projection weights are pre-scaled on the host by
    A = scale * 2^7 * log2(e), so the QK matmul's PSUM output is already
    the bf16-Schraudolph-affine a*x (x = scaled score).
  - ACT halves: nc.scalar.activation(Exp, scale=ln2/2^7) reads PSUM
    directly and produces the TRUE exp in bf16 (the affine fold cancels
    exactly).
  - DVE halves: one tensor_scalar add of B=127*2^7-C with int16 output
    into a bf16 tile; the bitcast is the classic fast-exp (max rel err
    ~3%, washes out to ~7e-3 on the final output after softmax
    normalization + AV averaging). One DVE op per element - same cost as
    a plain copy. E and v_aug are bf16, so the AV matmul runs bf16 x bf16
    (1 cyc/row, no fp32r roundedness rule).
Softmax skips max-subtraction: scaled scores are ~N(0,1), safe in fp32 exp.

PSUM budget (8 banks x 2KB): s half-chunk tiles 5x1 bank + pots-pair 2x1
+ scratch 1. The two heads' AV accumulate into ONE bank at partitions
0-32 and 64-96; start=True only on each head's first AV (the pending-zero
clear is per partition). pots is double-buffered so the next iteration's
AVs never wait on the previous normalization (the noexp timing probe
showed that serialization dominating the wall clock).

Normalization per iteration: 2 reciprocals -> one 2-row selector matmul
broadcasts both heads' 1/den to [64, 512] -> 2 fused scalar_tensor_tensor
(O * bc) ops write normalized O^T in f32r.
"""

import os
from contextlib import ExitStack

import numpy as np

import concourse.bacc as bacc
import concourse.bass as bass
import concourse.mybir as mybir
import concourse.tile as tile
from concourse.bass_utils import run_bass_kernel_spmd

B, N, C = 4, 2048, 256
H, D = 8, 32
P = 128
QH = N // 2              # query rows per core
SCALE = float(D) ** -0.5
NCORES = 8
NCH = N // P             # 16 k-chunks

F32 = mybir.dt.float32
F32R = mybir.dt.float32r
BF16 = mybir.dt.bfloat16
I16 = mybir.dt.int16
EXP = mybir.ActivationFunctionType.Exp
ADD = mybir.AluOpType.add
MUL = mybir.AluOpType.mult

LOG2E = 1.4426950408889634
# Host folds A_FOLD into wq; PSUM scores are then a*x with a = 2^7*log2(e)
# per unit of scaled score x = SCALE * (q.k)  (2^7: bf16-Schraudolph).
A_FOLD = SCALE * LOG2E * float(2**7)
G_ACT = 1.0 / (LOG2E * float(2**7))  # exp(psum * G_ACT) == true exp(x)
C_CORR = float(os.environ.get("BASS_ATTN_CCORR", "300000"))
B_DVE = 127.0 * float(2**7) - C_CORR / float(2**16)

# How many of the 32 half-chunks per iteration use the DVE fast-exp path
# (rest use ACT true exp).
NDVE = int(os.environ.get("BASS_ATTN_NDVE", "13"))
DVE_CHUNKS = set(round((i + 0.5) * NCH / NDVE) % NCH for i in range(NDVE))

# Timing amplification for the local harness (repeat attention+proj body).
REPS = int(os.environ.get("BASS_ATTN_REPS", "1"))
REPS_MODE = os.environ.get("BASS_ATTN_REPS_MODE", "loop")  # "loop" | "unroll"
TRAIL = int(os.environ.get("BASS_ATTN_TRAIL", "1"))  # AV trails its chunk by this many ticks
# Timing-decomposition variants (break correctness, timing only):
#   "" normal; "justs" QK only (tiny keep-alive read, no exp/AV/norm);
#   "noexp" QK+AV with a constant E (no exp work on ACT/DVE)
VARIANT = os.environ.get("BASS_ATTN_VARIANT", "")


def _emit(tc, xT, xTq, wall, pb, y):
    nc = tc.nc
    with ExitStack() as ctx:
        singles = ctx.enter_context(tc.tile_pool(name="singles", bufs=1))
        epool = ctx.enter_context(tc.tile_pool(name="epool", bufs=8))
        small = ctx.enter_context(tc.tile_pool(name="small", bufs=4))
        ypool = ctx.enter_context(tc.tile_pool(name="ypool", bufs=3))
        # PSUM (8 banks): s half-chunk tiles 5x1 + pots-pair 2x1 + scratch 1x1
        ps = ctx.enter_context(tc.tile_pool(name="ps", bufs=5, space="PSUM"))
        po = ctx.enter_context(tc.tile_pool(name="po", bufs=2, space="PSUM"))
        bcp = ctx.enter_context(tc.tile_pool(name="bcp", bufs=1, space="PSUM"))

        # ---- input loads + fp32r rounding (staged) -----------------------
        # all four weight matrices arrive packed in one dram tensor (one DMA,
        # one rounding copy); order: wk, wq, wv, pw
        wall_ld = singles.tile([P, 2, 4 * C], F32, tag="wall_ld", name="wall_ld")
        wall_sb = singles.tile([P, 2, 4 * C], F32R, tag="wall", name="wall_sb")
        wall4 = wall_sb[:].rearrange("p c (w n) -> p c w n", n=C)
        wk_sb = wall4[:, :, 0]
        wq_sb = wall4[:, :, 1]
        wv_sb = wall4[:, :, 2]
        pw_sb = wall4[:, :, 3]
        # x^T loads staged + rounded in 512-column blocks so the first qkv
        # matmuls (and the attention stream behind them) start early.
        xT_ld = singles.tile([P, 2, N], F32, tag="xT_ld", name="xT_ld")
        xT_sb = singles.tile([P, 2, N], F32R, tag="xT", name="xT_sb")
        xT_r = xT.rearrange("(c p) n -> p c n", p=P)

        def load_x_block(nb, eng, dma=None):
            sl = (slice(None), slice(None), slice(512 * nb, 512 * nb + 512))
            (dma or nc.sync).dma_start(xT_ld[sl], xT_r[sl])
            if eng == "act":
                nc.scalar.activation(xT_sb[sl], xT_ld[sl], mybir.ActivationFunctionType.Copy)
            else:
                nc.vector.tensor_copy(xT_sb[sl], xT_ld[sl])

        # wk's slice of the packed weights lands via its own small DMA so the
        # first k-projection matmul isn't gated on the full 8KB/partition
        # weight transfer (saved ~4us of PE prologue idle in the cost model).
        wall_r = wall.rearrange("(c p) n -> p c n", p=P)
        nc.sync.dma_start(wall_ld[:, :, 0 : 2 * C], wall_r[:, :, 0 : 2 * C])
        load_x_block(0, "dve", dma=nc.scalar)
        # round the k/q projection slices first: they gate the first matmuls
        nc.vector.tensor_copy(wall_sb[:, :, 0:C], wall_ld[:, :, 0:C])
        nc.scalar.activation(
            wall_sb[:, :, C : 2 * C], wall_ld[:, :, C : 2 * C],
            mybir.ActivationFunctionType.Copy,
        )
        xTq_ld = singles.tile([P, 2, QH], F32, tag="xTq_ld", name="xTq_ld")
        xTq_sb = singles.tile([P, 2, QH], F32R, tag="xTq", name="xTq_sb")
        xTq_r = xTq.rearrange("(c p) n -> p c n", p=P)

        def load_xq_block(nb, eng, dma=None):
            sl = (slice(None), slice(None), slice(512 * nb, 512 * nb + 512))
            (dma or nc.sync).dma_start(xTq_ld[sl], xTq_r[sl])
            if eng == "act":
                nc.scalar.activation(xTq_sb[sl], xTq_ld[sl], mybir.ActivationFunctionType.Copy)
            else:
                nc.vector.tensor_copy(xTq_sb[sl], xTq_ld[sl])

        load_xq_block(0, "act")
        load_x_block(1, "act")
        # wv/pw arrive after the iteration-0 gating loads; the first v
        # consumers (AV chunk tt) have slack until tick tt+1.
        nc.sync.dma_start(wall_ld[:, :, 2 * C : 4 * C], wall_r[:, :, 2 * C : 4 * C])
        nc.scalar.activation(
            wall_sb[:, :, 2 * C : 4 * C], wall_ld[:, :, 2 * C : 4 * C],
            mybir.ActivationFunctionType.Copy,
        )
        pb_sb = singles.tile([P, C], F32, tag="pb")
        nc.sync.dma_start(
            pb_sb[:],
            bass.AP(tensor=pb.tensor, offset=pb.offset, ap=[[0, P]] + list(pb.ap)),
        )

        # ---- qkv projection emitters ------------------------------------
        # q^T/k^T stacks: chunk cc holds heads 4cc..4cc+3 at rows 32*(h%4).
        qT_sb = singles.tile([P, 2, QH], F32R, tag="qT")
        kT_sb = singles.tile([P, 2, N], F32R, tag="kT")
        # v_aug: [token-tile, head-major (v_h | 1)] for AV + denominator.
        # bf16: the AV matmul runs bf16 x bf16 (1 cyc/row at any free size,
        # no fp32r roundedness rule to satisfy).
        vA_sb = singles.tile([P, NCH, H * (D + 1)], BF16, tag="vA")
        vA4 = vA_sb[:].rearrange("p t (h a) -> p t h a", a=D + 1)
        nc.vector.memset(vA4[:, :, :, D], 1.0)

        def emit_kqT(w_sb, x_sb, out_sb, cc, nb, eng):
            pk = bcp.tile([P, 512], F32, tag="bc", name="pk")
            for ci in range(2):
                nc.tensor.matmul(
                    pk[:],
                    lhsT=w_sb[:, ci, 128 * cc : 128 * cc + 128],
                    rhs=x_sb[:, ci, 512 * nb : 512 * nb + 512],
                    start=(ci == 0),
                    stop=(ci == 1),
                )
            dst = out_sb[:, cc, 512 * nb : 512 * nb + 512]
            if eng == "act":
                nc.scalar.activation(dst, pk[:], mybir.ActivationFunctionType.Copy)
            else:
                nc.vector.tensor_copy(dst, pk[:])

        def emit_v(tt, eng):
            pv = bcp.tile([P, 512], F32, tag="bc", name="pv")
            for ci in range(2):
                nc.tensor.matmul(
                    pv[:, 0:256],
                    lhsT=xT_sb[:, ci, 128 * tt : 128 * tt + 128],
                    rhs=wv_sb[:, ci, :],
                    start=(ci == 0),
                    stop=(ci == 1),
                )
            dst = vA4[:, tt, :, 0:D]
            src = pv[:, 0:256].rearrange("p (h d) -> p h d", d=D)
            if eng == "act":
                nc.scalar.activation(dst, src, mybir.ActivationFunctionType.Copy)
            else:
                nc.vector.tensor_copy(dst, src)

        # ---- attention helpers ------------------------------------------
        # selector for the 1/den broadcast: bc[0:32]=rcpA, bc[32:64]=rcpB.
        # Engine APs may only start at partitions 0/32/64/96, so the two
        # reciprocal rows live at partitions 0 and 32 of a persistent
        # [64, 512] tile (other partitions zeroed once: the selector matmul
        # contracts over all 64).
        et_const = None
        if VARIANT == "noexp":
            et_const = singles.tile([P, 1024], BF16, tag="etc")
            nc.vector.memset(et_const[:], 0.5)
        keep_sb = None
        if VARIANT in ("justs", "noexp"):
            keep_sb = singles.tile([1, 2 * NCH * 32], F32, tag="keep")
        self_f = singles.tile([64, 64], BF16, tag="self")
        nc.vector.memset(self_f[:], 0.0)
        nc.vector.memset(self_f[0:1, 0:32], 1.0)
        nc.vector.memset(self_f[32:33, 32:64], 1.0)
        rcp2 = singles.tile([64, 512], BF16, tag="rcp2")
        nc.vector.memset(rcp2[:], 0.0)
        OT_sb = singles.tile([P, 2, QH], F32R, tag="OT")

        def emit_proj_qt(qb, qt):
            tq = 4 * qb + qt
            py = bcp.tile([P, 512], F32, tag="bc", name="py")
            for ci in range(2):
                nc.tensor.matmul(
                    py[:, 0:256],
                    lhsT=OT_sb[:, ci, 128 * tq : 128 * tq + 128],
                    rhs=pw_sb[:, ci, :],
                    start=(ci == 0),
                    stop=(ci == 1),
                )
            ysb = ypool.tile([P, C], F32, tag="y", name="ysb")
            nc.vector.tensor_add(ysb[:], py[:, 0:256], pb_sb[:])
            nc.sync.dma_start(y[128 * tq : 128 * tq + 128, :], ysb[:])

        def emit_av(pots, hp, ets, ch, first):
            # pots: [97, 512] pair tile; head A rows 0:33, head B rows 64:97.
            # start=True on each head's first matmul only (the pending-zero
            # clear is per partition, so the heads don't clobber each other).
            for e in range(2):
                h = 2 * hp + e
                et = ets[e] if isinstance(ets, (list, tuple)) else ets
                src_ap = et[:] if et.shape[-1] == 512 else et[:, 512 * e : 512 * e + 512]
                nc.tensor.matmul(
                    pots[64 * e : 64 * e + D + 1, :],
                    lhsT=vA_sb[:, ch, (D + 1) * h : (D + 1) * (h + 1)],
                    rhs=src_ap,
                    start=first,
                    stop=(ch == NCH - 1),
                    skip_group_check=True,
                )

        def emit_body():
            # ---- prefix: just enough qkv for iteration 0's first chunks ------
            emit_kqT(wk_sb, xT_sb, kT_sb, 0, 0, "dve")
            emit_kqT(wq_sb, xTq_sb, qT_sb, 0, 0, "act")
            for tt in range(4):
                emit_v(tt, "act" if tt % 2 else "dve")
            # x blocks 2-3 and the second q-half load late: their staging
            # copies queue behind the iteration-0 gating copies above, and
            # their first consumers (v8+/kT(0,2)+, qb=1) have ticks of slack.
            for nb in range(2, 4):
                load_x_block(nb, "act" if nb % 2 else "dve")
            load_xq_block(1, "dve")
            # remaining qkv work, spread one item per chunk tick; deadlines:
            # v_tt by tick tt, kT(0,nb) before tick 4nb, cc=1 before tick 32.
            worklist = [
                lambda: emit_v(4, "act"),
                lambda: emit_kqT(wk_sb, xT_sb, kT_sb, 0, 1, "dve"),
                lambda: emit_v(5, "act"),
                lambda: emit_v(6, "act"),
                lambda: emit_v(7, "dve"),
                lambda: emit_kqT(wk_sb, xT_sb, kT_sb, 0, 2, "act"),
                lambda: emit_v(8, "act"),
                lambda: emit_v(9, "dve"),
                lambda: emit_v(10, "act"),
                lambda: emit_kqT(wk_sb, xT_sb, kT_sb, 0, 3, "act"),
                lambda: emit_v(11, "dve"),
                lambda: emit_v(12, "act"),
                lambda: emit_v(13, "act"),
                lambda: emit_v(14, "dve"),
                lambda: emit_v(15, "act"),
                lambda: emit_kqT(wq_sb, xTq_sb, qT_sb, 0, 1, "act"),
                lambda: emit_kqT(wk_sb, xT_sb, kT_sb, 1, 0, "dve"),
                lambda: emit_kqT(wk_sb, xT_sb, kT_sb, 1, 1, "act"),
                lambda: emit_kqT(wk_sb, xT_sb, kT_sb, 1, 2, "act"),
                lambda: emit_kqT(wk_sb, xT_sb, kT_sb, 1, 3, "dve"),
                lambda: emit_kqT(wq_sb, xTq_sb, qT_sb, 1, 0, "act"),
                lambda: emit_kqT(wq_sb, xTq_sb, qT_sb, 1, 1, "act"),
            ]

            # ---- attention main loop -----------------------------------------
            # Per chunk: two half-chunk s tiles (one PSUM bank each, one per
            # head) so ACT and DVE exp different banks concurrently; the
            # engine split is a per-half pattern (~19 ACT / 13 DVE per iter).
            # AV trails its chunk by one tick. pots double-buffered: the next
            # iteration's AVs never wait on the previous normalization.
            deferred = []
            body_reps = REPS if (REPS > 1 and REPS_MODE == "unroll") else 1
            its = [
                (qb, hp)
                for _ in range(body_reps)
                for qb in range(QH // 512)
                for hp in range(H // 2)
            ]
            n_halves = 2 * NCH
            dve_halves = set(
                round((i + 0.5) * n_halves / NDVE) % n_halves for i in range(NDVE)
            )
            pots = None
            for it_i, (qb, hp) in enumerate(its):
                pots = po.tile([97, 512], F32, tag="o", name="pots")
                pend = []
                for ch in range(NCH):
                    ets = []
                    for e in range(2):
                        h = 2 * hp + e
                        r, cc = 32 * (h % 4), h // 4
                        s = ps.tile([P, 512], F32, tag="s", name="s")
                        nc.tensor.matmul(
                            s[:],
                            lhsT=kT_sb[r : r + 32, cc, 128 * ch : 128 * ch + 128],
                            rhs=qT_sb[r : r + 32, cc, 512 * qb : 512 * qb + 512],
                            start=True,
                            stop=True,
                            tile_position=(r, 0),
                        )
                        if VARIANT in ("justs", "noexp"):
                            nc.vector.tensor_copy(
                                keep_sb[:, 32 * (2 * ch + e) : 32 * (2 * ch + e) + 32],
                                s[0:1, 0:32],
                            )
                            et = et_const
                        else:
                            et = epool.tile([P, 512], BF16, tag="E", name="et")
                            if (2 * ch + e) in dve_halves:
                                nc.vector.tensor_scalar(
                                    out=et[:].bitcast(I16), in0=s[:], scalar1=B_DVE,
                                    scalar2=None, op0=ADD,
                                )
                            else:
                                nc.scalar.activation(et[:], s[:], EXP, scale=G_ACT)
                        ets.append(et)
                    if VARIANT == "justs":
                        if deferred:
                            deferred.pop(0)()
                        elif worklist:
                            worklist.pop(0)()
                        continue
                    pend.append((ets, ch))
                    if len(pend) > TRAIL:
                        pets, pch = pend.pop(0)
                        emit_av(pots, hp, pets, pch, pch == 0)
                    if deferred:
                        deferred.pop(0)()
                    elif worklist:
                        worklist.pop(0)()
                if VARIANT == "justs":
                    pots = "dummy"  # epilogue skipped below
                    continue
                for pets, pch in pend:
                    emit_av(pots, hp, pets, pch, pch == 0)
                if (qb, hp) != its[-1]:
                    # normalization of THIS iteration's pots, deferred into
                    # the start of the next iteration (one item per tick).
                    def d_recip(row, prow, pp):
                        with nc.allow_low_precision(reason="1/den in bf16 feeds the broadcast matmul; 0.4% norm error is within tolerance"):
                            nc.vector.reciprocal(rcp2[row : row + 1, :], pp[prow : prow + 1, :])

                    deferred.append(lambda pp=pots: d_recip(0, D, pp))
                    deferred.append(lambda pp=pots: d_recip(32, 64 + D, pp))
                    bc_holder = []

                    def d_bc(hold=bc_holder):
                        bc = bcp.tile([64, 512], F32, tag="bc", name="bcn")
                        nc.tensor.matmul(
                            bc[:], lhsT=self_f[:], rhs=rcp2[:], start=True, stop=True
                        )
                        bcs = small.tile([64, 512], F32, tag="bcs", name="bcs")
                        nc.vector.tensor_copy(bcs[:], bc[:])
                        hold.append(bcs)

                    deferred.append(d_bc)

                    def d_stt(e, pp=pots, hold=bc_holder, qb_=qb, hp_=hp):
                        h = 2 * hp_ + e
                        r, cc = 32 * (h % 4), h // 4
                        nc.vector.scalar_tensor_tensor(
                            out=OT_sb[r : r + 32, cc, 512 * qb_ : 512 * qb_ + 512],
                            in0=pp[64 * e : 64 * e + D, :],
                            scalar=1.0,
                            in1=hold[0][32 * e : 32 * e + 32, :],
                            op0=MUL,
                            op1=MUL,
                        )

                    deferred.append(lambda: d_stt(0))
                    deferred.append(lambda: d_stt(1))
                    if hp == H // 2 - 1:
                        for qt in range(4):
                            deferred.append(lambda a=qb, b=qt: emit_proj_qt(a, b))
            for act in deferred:
                act()
            # fast epilogue for the final head pair: scratch from the now-idle
            # s pool; plain-fp32 broadcast matmul (PE is idle here).
            l_qb, l_hp = its[-1]
            with nc.allow_low_precision(reason="1/den in bf16 (see d_recip)"):
                nc.vector.reciprocal(rcp2[0:1, :], pots[D : D + 1, :])
                nc.vector.reciprocal(rcp2[32:33, :], pots[64 + D : 64 + D + 1, :])
            bce = ps.tile([P, 512], F32, tag="s", name="bce")
            nc.tensor.matmul(
                bce[0:64, :], lhsT=self_f[:], rhs=rcp2[:], start=True, stop=True
            )
            bces = small.tile([64, 512], F32, tag="bcs", name="bces")
            nc.scalar.activation(bces[:], bce[0:64, :], mybir.ActivationFunctionType.Copy)
            for e in range(2):
                h = 2 * l_hp + e
                r, cc = 32 * (h % 4), h // 4
                nc.vector.scalar_tensor_tensor(
                    out=OT_sb[r : r + 32, cc, 512 * l_qb : 512 * l_qb + 512],
                    in0=pots[64 * e : 64 * e + D, :],
                    scalar=1.0,
                    in1=bces[32 * e : 32 * e + 32, :],
                    op0=MUL,
                    op1=MUL,
                )
            for qt in range(4):
                tq = 4 * l_qb + qt
                py = ps.tile([P, 512], F32, tag="s", name="pyT")
                for ci in range(2):
                    nc.tensor.matmul(
                        py[:, 0:256],
                        lhsT=OT_sb[:, ci, 128 * tq : 128 * tq + 128],
                        rhs=pw_sb[:, ci, :],
                        start=(ci == 0),
                        stop=(ci == 1),
                    )
                ysb = ypool.tile([P, C], F32, tag="y", name="ysb")
                nc.vector.tensor_add(ysb[:], py[:, 0:256], pb_sb[:])
                nc.sync.dma_start(y[128 * tq : 128 * tq + 128, :], ysb[:])

        if REPS == 1 or REPS_MODE == "unroll":
            emit_body()
        else:
            with tc.For_i(0, REPS, 1):
                emit_body()


_NC = None
_RUNNER = None


def _get_runner():
    """Cached SPMD runner: builds the jitted shard_map executable once so warm
    kernel() calls skip JAX retracing/compilation (run_bass_kernel_spmd builds
    a fresh closure per call, which always misses the jit cache)."""
    global _RUNNER
    if _RUNNER is not None:
        return _RUNNER
    import jax
    from jax.sharding import Mesh, PartitionSpec
    from jax.experimental.shard_map import shard_map
    from concourse import bass2jax, mybir as _mb

    nc = _get_nc()
    bass2jax.install_neuronx_cc_hook()

    assert nc.dbg_addr is None
    partition_name = nc.partition_id_tensor.name if nc.partition_id_tensor else None
    in_names, out_names, out_avals = [], [], []
    for alloc in nc.m.functions[0].allocations:
        if not isinstance(alloc, _mb.MemoryLocationSet):
            continue
        name = alloc.memorylocations[0].name
        if alloc.kind == "ExternalInput":
            if name != partition_name:
                in_names.append(name)
        elif alloc.kind == "ExternalOutput":
            out_names.append(name)
            out_avals.append(
                jax.core.ShapedArray(tuple(alloc.tensor_shape), _mb.dt.np(alloc.dtype))
            )
    n_params = len(in_names)
    n_outs = len(out_avals)
    all_names = in_names + out_names
    if partition_name is not None:
        all_names = all_names + [partition_name]

    def _body(*args):
        operands = list(args)
        if partition_name is not None:
            operands.append(bass2jax.partition_id_tensor())
        outs = bass2jax._bass_exec_p.bind(
            *operands,
            out_avals=tuple(out_avals),
            in_names=tuple(all_names),
            out_names=tuple(out_names),
            lowering_input_output_aliases=(),
            sim_require_finite=True,
            sim_require_nnan=True,
            nc=nc,
        )
        return tuple(outs)

    devices = jax.devices()[:NCORES]
    mesh = Mesh(np.asarray(devices), ("core",))
    sharded = jax.jit(
        shard_map(
            _body,
            mesh=mesh,
            in_specs=(PartitionSpec("core"),) * (n_params + n_outs),
            out_specs=(PartitionSpec("core"),) * n_outs,
            check_rep=False,
        ),
        donate_argnums=tuple(range(n_params, n_params + n_outs)),
        keep_unused=True,
    )

    def run(in_maps):
        concat_in = [
            np.concatenate([np.asarray(m[nm]) for m in in_maps], axis=0)
            for nm in in_names
        ]
        concat_zeros = [
            np.zeros((NCORES * a.shape[0], *a.shape[1:]), a.dtype) for a in out_avals
        ]
        out_arrs = sharded(*concat_in, *concat_zeros)
        return [
            {
                nm: np.asarray(out_arrs[i]).reshape(NCORES, *out_avals[i].shape)[c]
                for i, nm in enumerate(out_names)
            }
            for c in range(NCORES)
        ]

    _RUNNER = run
    return run


def _get_nc():
    global _NC
    if _NC is None:
        nc = bacc.Bacc("TRN2", target_bir_lowering=False, debug=False, num_devices=1)
        xT = nc.dram_tensor("xT", [C, N], F32, kind="ExternalInput").ap()
        xTq = nc.dram_tensor("xTq", [C, QH], F32, kind="ExternalInput").ap()
        wall = nc.dram_tensor("wall", [C, 4 * C], F32, kind="ExternalInput").ap()
        pb = nc.dram_tensor("pb", [C], F32, kind="ExternalInput").ap()
        y = nc.dram_tensor("y", [QH, C], F32, kind="ExternalOutput").ap()
        with tile.TileContext(nc) as tc:
            _emit(tc, xT, xTq, wall, pb, y)
        nc.finalize()
        _NC = nc
    return _NC


def kernel(x, qkv_w, proj_w, proj_b):
    x = np.asarray(x, dtype=np.float32)
    qkv_w = np.asarray(qkv_w, dtype=np.float32)
    proj_w = np.asarray(proj_w, dtype=np.float32)
    proj_b = np.asarray(proj_b, dtype=np.float32)

    nc = _get_nc()
    wall = np.ascontiguousarray(
        np.stack(
            [
                qkv_w[C : 2 * C].T,                    # wk
                qkv_w[0:C].T * np.float32(A_FOLD),     # wq (Schraudolph fold)
                qkv_w[2 * C : 3 * C].T,                # wv
                proj_w.T,                              # pw
            ],
            axis=1,
        ).reshape(C, 4 * C)
    )

    in_maps = []
    for c in range(NCORES):
        b, qh = c // 2, c % 2
        xT = np.ascontiguousarray(x[b].T)
        in_maps.append(
            {
                "xT": xT,
                "xTq": np.ascontiguousarray(xT[:, qh * QH : (qh + 1) * QH]),
                "wall": wall,
                "pb": proj_b,
            }
        )
    results = _get_runner()(in_maps)
    out = np.empty((B, N, C), np.float32)
    for c in range(NCORES):
        b, qh = c // 2, c % 2
        out[b, qh * QH : (qh + 1) * QH] = results[c]["y"]
    return out
